# revision 2
# baseline (speedup 1.0000x reference)
"""Edge dot-product kernel (gnn_message_passing) for 8 Trainium2 NeuronCores.

out[e] = dot(x[senders[e]], x[receivers[e]]),  x: [100000, 32] f32,
senders/receivers: [2500000] int64, out: [2500000] f32.

Strategy:
  - Shard edges across the 8 cores (contiguous slices), replicate x.
  - On host, per core: bucket edges by (sender_range, receiver_range) where
    the 100000 nodes are split into 4 ranges of 25000.  Within a bucket both
    gathers address a single 25000-node window, so indices fit in int16 as
    required by the InstDMAGatherAnt SWDGE gather instruction.
  - x is stored in DRAM padded to a 256-byte row stride (the gather's stride
    granularity); each descriptor reads only the 128-byte payload row.
  - On device, per bucket: dma_gather sender rows and receiver rows into
    [128 edges, nblk, 32] SBUF tiles, multiply elementwise and reduce the
    feature axis on the vector engine, DMA the [128, nblk] dots out.
  - Host inverse-permutes the bucket-sorted dots back to edge order.
"""

import os
import sys
import types
import contextlib
import ctypes

import numpy as np

from concourse import bacc, mybir
import concourse.tile as tile
import concourse.ap_utils as ap_utils
from concourse.bass import exact_div, round_up_to_multiple
from concourse.bass_utils import run_bass_kernel_spmd

N_NODES = 100000
D_FEAT = 32
N_EDGES = 2500000
NCORES = 8
EPC = N_EDGES // NCORES          # 312500 edges per core
NRANGE = 4
RANGE = 25000                    # nodes per int16-addressable window
NBUCKET = NRANGE * NRANGE        # 16 (sender_range, receiver_range) buckets
STEP = 64                        # padded x row: 64 f32 = 256B stride
MAX_GATHER = 8192                # SWDGE descriptor-ring capacity per instruction
DEFAULT_CAP = 20480              # per-bucket edge capacity (mean 19531, +7 sigma)

LAST_EXEC_NS = None              # set when KERNEL_TRACE=1


def _raw_dma_gather(eng, out_ap, in_ap, idxs_ap, num_idxs, elem_size, elem_step):
    """bass dma_gather minus the elem_size%256 assert (transpose-only
    restriction applied too broadly); payload may be any size, stride must
    still be a 256B multiple."""
    assert idxs_ap.dtype == mybir.dt.int16
    assert in_ap.dtype == out_ap.dtype
    assert ap_utils.ap_is_contiguous(in_ap.ap[1:])
    assert ap_utils.ap_is_contiguous(out_ap.ap[1:])
    assert ap_utils.ap_is_contiguous(idxs_ap.ap[1:])
    assert in_ap.ap[-1][1] == out_ap.ap[-1][1] == elem_size
    assert out_ap.ap[0][1] * out_ap.ap[1][1] == round_up_to_multiple(num_idxs, 128)
    assert in_ap.ap[0][0] == elem_step
    stride_bytes_256 = exact_div(elem_step * mybir.dt.size(in_ap.dtype), 256)
    assert stride_bytes_256 < 256
    _in_ap = eng.lower_ap_dma(in_ap, for_custom_bir_dma=True)
    _idxs_ap = eng.lower_ap(idxs_ap)
    _out_ap = eng.lower_ap(out_ap)
    return eng.add_instruction(
        mybir.InstDMAGatherAnt(
            name=eng.bass.get_next_instruction_name(),
            ins=[*_in_ap, _idxs_ap, eng.lower_val_access(eng.to_reg(num_idxs))],
            outs=[_out_ap],
            transpose=False,
            num_idxs=num_idxs,
            elem_size=elem_size,
            stride_bytes_256=stride_bytes_256,
            gen_mode=0,
            single_packet=False,
            queue_num=0,
            sbuf_tokens_per_rank=0,
            sbuf_free_dim_per_rank=0,
            sbuf_free_dim_pad_per_rank=0,
            sbuf_byte_offset=0,
        )
    )


def _chunks(cap):
    offs = []
    off = 0
    while off < cap:
        n = min(MAX_GATHER, cap - off)
        offs.append((off, n))
        off += n
    return offs


_program_cache = {}


def _build_program(cap):
    if cap in _program_cache:
        return _program_cache[cap]
    nc = bacc.Bacc("TRN2", target_bir_lowering=False, debug=False, num_devices=NCORES)
    x_d = nc.dram_tensor("x", [N_NODES, STEP], mybir.dt.float32, kind="ExternalInput")
    s_d = nc.dram_tensor("sidx", [NBUCKET, 128, cap // 16], mybir.dt.int16, kind="ExternalInput")
    r_d = nc.dram_tensor("ridx", [NBUCKET, 128, cap // 16], mybir.dt.int16, kind="ExternalInput")
    o_d = nc.dram_tensor("out", [NBUCKET, 128, cap // 128], mybir.dt.float32, kind="ExternalOutput")
    with tile.TileContext(nc) as tc:
        with tc.tile_pool(name="idx", bufs=3) as ip, \
             tc.tile_pool(name="g", bufs=4) as gp, \
             tc.tile_pool(name="o", bufs=4) as op:
            for b in range(NBUCKET):
                sr, rr = b // NRANGE, b % NRANGE
                sit = ip.tile([128, cap // 16], mybir.dt.int16, tag="si")
                rit = ip.tile([128, cap // 16], mybir.dt.int16, tag="ri")
                nc.sync.dma_start(out=sit[:], in_=s_d[b])
                nc.sync.dma_start(out=rit[:], in_=r_d[b])
                xs = x_d[sr * RANGE:(sr + 1) * RANGE, :D_FEAT]
                xr = x_d[rr * RANGE:(rr + 1) * RANGE, :D_FEAT]
                for off, n in _chunks(cap):
                    gs = gp.tile([128, n // 128, D_FEAT], mybir.dt.float32, tag="gs")
                    gr = gp.tile([128, n // 128, D_FEAT], mybir.dt.float32, tag="gr")
                    _raw_dma_gather(nc.gpsimd, gs[:], xs,
                                    sit[:, off // 16:(off + n) // 16], n, D_FEAT, STEP)
                    _raw_dma_gather(nc.gpsimd, gr[:], xr,
                                    rit[:, off // 16:(off + n) // 16], n, D_FEAT, STEP)
                    nc.vector.tensor_tensor(out=gs[:], in0=gs[:], in1=gr[:],
                                            op=mybir.AluOpType.mult)
                    ot = op.tile([128, n // 128], mybir.dt.float32, tag="o")
                    nc.vector.tensor_reduce(out=ot[:], in_=gs[:],
                                            axis=mybir.AxisListType.X,
                                            op=mybir.AluOpType.add)
                    nc.sync.dma_start(out=o_d[b, :, off // 128:(off + n) // 128],
                                      in_=ot[:])
    nc.compile()
    _program_cache[cap] = nc
    return nc


def _install_profile_hook():
    """Register the axon NTFF profile hook (the image's antenv lacks the
    axon_hooks module the boot path would normally use)."""
    import antenv
    if "antenv.axon_hooks" in sys.modules:
        return True
    mod = types.ModuleType("antenv.axon_hooks")
    _hook = [None]
    mod.set_axon_ntff_profile_hook = lambda h: _hook.__setitem__(0, h)
    mod.get_axon_ntff_profile_hook = lambda: _hook[0]
    sys.modules["antenv.axon_hooks"] = mod
    antenv.axon_hooks = mod
    try:
        if "/root/.axon_site" not in sys.path:
            sys.path.insert(0, "/root/.axon_site")
        from trn_agent_boot.trn_boot import _ntff_profile_via_ctypes
        mod.set_axon_ntff_profile_hook(_ntff_profile_via_ctypes("/opt/axon/libaxon_pjrt.so"))
        return True
    except Exception:
        return False


def _wrap16(arr2d):
    """[B, cap] -> [B, 128, cap//16]: index j of each row at [j%16, j//16],
    replicated across the 8 groups of 16 partitions (one per Q7 core)."""
    b, cap = arr2d.shape
    w = arr2d.reshape(b, cap // 16, 16).transpose(0, 2, 1)  # [B, 16, cap//16]
    return np.tile(w, (1, 8, 1)).copy()


def kernel(x, senders, receivers):
    global LAST_EXEC_NS
    x = np.ascontiguousarray(np.asarray(x, dtype=np.float32))
    s = np.asarray(senders).astype(np.int32)
    r = np.asarray(receivers).astype(np.int32)
    n_edges = s.shape[0]
    epc = -(-n_edges // NCORES)

    xp = np.zeros((N_NODES, STEP), np.float32)
    xp[:, :D_FEAT] = x

    percore = []
    max_count = 0
    for c in range(NCORES):
        sc = s[c * epc:(c + 1) * epc]
        rc = r[c * epc:(c + 1) * epc]
        bucket = (sc // RANGE) * NRANGE + (rc // RANGE)
        order = np.argsort(bucket, kind="stable")
        counts = np.bincount(bucket, minlength=NBUCKET)
        max_count = max(max_count, int(counts.max()))
        percore.append((sc, rc, order, counts))

    cap = max(DEFAULT_CAP, round_up_to_multiple(max_count, 1024))
    nc = _build_program(cap)

    in_maps = []
    for c in range(NCORES):
        sc, rc, order, counts = percore[c]
        sl = np.zeros((NBUCKET, cap), np.int16)
        rl = np.zeros((NBUCKET, cap), np.int16)
        ss = (sc[order] % RANGE).astype(np.int16)
        rs = (rc[order] % RANGE).astype(np.int16)
        bounds = np.concatenate(([0], np.cumsum(counts)))
        for b in range(NBUCKET):
            cnt = counts[b]
            sl[b, :cnt] = ss[bounds[b]:bounds[b + 1]]
            rl[b, :cnt] = rs[bounds[b]:bounds[b + 1]]
        in_maps.append({"x": xp, "sidx": _wrap16(sl), "ridx": _wrap16(rl)})

    trace = bool(os.environ.get("KERNEL_TRACE")) and _install_profile_hook()
    res = run_bass_kernel_spmd(nc, in_maps, list(range(NCORES)), trace=trace)
    LAST_EXEC_NS = res.exec_time_ns

    out = np.empty(epc * NCORES, np.float32)
    for c in range(NCORES):
        sc, rc, order, counts = percore[c]
        dev = res.results[c]["out"]  # [NBUCKET, 128, cap//128]
        vals = np.concatenate(
            [dev[b].T.ravel()[:counts[b]] for b in range(NBUCKET)])
        oc = out[c * epc:(c + 1) * epc]
        oc[order] = vals
    return out[:n_edges]


# revision 3
# speedup vs baseline: 3.5196x; 3.5196x over previous
"""Edge dot-product kernel (gnn_message_passing) for 8 Trainium2 NeuronCores.

out[e] = dot(x[senders[e]], x[receivers[e]]),  x: [100000, 32] f32,
senders/receivers: [2500000] int64, out: [2500000] f32.

Strategy:
  - Shard edges across the 8 cores (contiguous slices), replicate x.
  - On host, per core: bucket edges by (sender_range, receiver_range) where
    the 100000 nodes are split into 4 ranges of 25000.  Within a bucket both
    gathers address a single 25000-node window, so indices fit in int16 as
    required by the InstDMAGatherAnt SWDGE gather instruction.
  - x is stored in DRAM padded to a 256-byte row stride (the gather's stride
    granularity); each descriptor reads only the 128-byte payload row.
  - On device, per bucket: dma_gather sender rows and receiver rows into
    [128 edges, nblk, 32] SBUF tiles, multiply elementwise and reduce the
    feature axis on the vector engine, DMA the [128, nblk] dots out.
  - Host inverse-permutes the bucket-sorted dots back to edge order.
"""

import os
import sys
import types
import contextlib
import ctypes

import numpy as np

from concourse import bacc, mybir
import concourse.tile as tile
import concourse.ap_utils as ap_utils
from concourse.bass import exact_div, round_up_to_multiple
from concourse.bass_utils import run_bass_kernel_spmd

N_NODES = 100000
D_FEAT = 32
N_EDGES = 2500000
NCORES = 8
EPC = N_EDGES // NCORES          # 312500 edges per core
NRANGE = 4
RANGE = 25000                    # nodes per int16-addressable window
NBUCKET = NRANGE * NRANGE        # 16 (sender_range, receiver_range) buckets
STEP = 64                        # padded x row: 64 f32 = 256B stride
MAX_GATHER = 8192                # SWDGE descriptor-ring capacity per instruction
DEFAULT_CAP = 20480              # per-bucket edge capacity (mean 19531, +7 sigma)

LAST_EXEC_NS = None              # set when KERNEL_TRACE=1


def _raw_dma_gather(eng, out_ap, in_ap, idxs_ap, num_idxs, elem_size, elem_step, queue_num=0):
    """bass dma_gather minus the elem_size%256 assert (transpose-only
    restriction applied too broadly); payload may be any size, stride must
    still be a 256B multiple."""
    assert idxs_ap.dtype == mybir.dt.int16
    assert in_ap.dtype == out_ap.dtype
    assert ap_utils.ap_is_contiguous(in_ap.ap[1:])
    assert ap_utils.ap_is_contiguous(out_ap.ap[1:])
    assert ap_utils.ap_is_contiguous(idxs_ap.ap[1:])
    assert in_ap.ap[-1][1] == out_ap.ap[-1][1] == elem_size
    assert out_ap.ap[0][1] * out_ap.ap[1][1] == round_up_to_multiple(num_idxs, 128)
    assert in_ap.ap[0][0] == elem_step
    stride_bytes_256 = exact_div(elem_step * mybir.dt.size(in_ap.dtype), 256)
    assert stride_bytes_256 < 256
    _in_ap = eng.lower_ap_dma(in_ap, for_custom_bir_dma=True)
    _idxs_ap = eng.lower_ap(idxs_ap)
    _out_ap = eng.lower_ap(out_ap)
    return eng.add_instruction(
        mybir.InstDMAGatherAnt(
            name=eng.bass.get_next_instruction_name(),
            ins=[*_in_ap, _idxs_ap, eng.lower_val_access(eng.to_reg(num_idxs))],
            outs=[_out_ap],
            transpose=False,
            num_idxs=num_idxs,
            elem_size=elem_size,
            stride_bytes_256=stride_bytes_256,
            gen_mode=0,
            single_packet=False,
            queue_num=queue_num,
            sbuf_tokens_per_rank=0,
            sbuf_free_dim_per_rank=0,
            sbuf_free_dim_pad_per_rank=0,
            sbuf_byte_offset=0,
        )
    )


def _chunks(cap):
    offs = []
    off = 0
    while off < cap:
        n = min(MAX_GATHER, cap - off)
        offs.append((off, n))
        off += n
    return offs


_program_cache = {}


def _build_program(cap):
    _build_program.qctr = 0
    if cap in _program_cache:
        return _program_cache[cap]
    nc = bacc.Bacc("TRN2", target_bir_lowering=False, debug=False, num_devices=NCORES,
                   num_swdge_queues=4)
    x_d = nc.dram_tensor("x", [N_NODES, STEP], mybir.dt.float32, kind="ExternalInput")
    s_d = nc.dram_tensor("sidx", [NBUCKET, 128, cap // 16], mybir.dt.int16, kind="ExternalInput")
    r_d = nc.dram_tensor("ridx", [NBUCKET, 128, cap // 16], mybir.dt.int16, kind="ExternalInput")
    o_d = nc.dram_tensor("out", [NBUCKET, 128, cap // 128], mybir.dt.float32, kind="ExternalOutput")
    with tile.TileContext(nc) as tc:
        with tc.tile_pool(name="idx", bufs=3) as ip, \
             tc.tile_pool(name="g", bufs=4) as gp, \
             tc.tile_pool(name="o", bufs=4) as op:
            for b in range(NBUCKET):
                sr, rr = b // NRANGE, b % NRANGE
                sit = ip.tile([128, cap // 16], mybir.dt.int16, tag="si")
                rit = ip.tile([128, cap // 16], mybir.dt.int16, tag="ri")
                nc.sync.dma_start(out=sit[:], in_=s_d[b])
                nc.sync.dma_start(out=rit[:], in_=r_d[b])
                xs = x_d[sr * RANGE:(sr + 1) * RANGE, :D_FEAT]
                xr = x_d[rr * RANGE:(rr + 1) * RANGE, :D_FEAT]
                for off, n in _chunks(cap):
                    gs = gp.tile([128, n // 128, D_FEAT], mybir.dt.float32, tag="gs")
                    gr = gp.tile([128, n // 128, D_FEAT], mybir.dt.float32, tag="gr")
                    q = _build_program.qctr
                    _raw_dma_gather(nc.gpsimd, gs[:], xs,
                                    sit[:, off // 16:(off + n) // 16], n, D_FEAT, STEP,
                                    queue_num=q % 4)
                    _raw_dma_gather(nc.gpsimd, gr[:], xr,
                                    rit[:, off // 16:(off + n) // 16], n, D_FEAT, STEP,
                                    queue_num=(q + 1) % 4)
                    _build_program.qctr = q + 2
                    nc.vector.tensor_tensor(out=gs[:], in0=gs[:], in1=gr[:],
                                            op=mybir.AluOpType.mult)
                    ot = op.tile([128, n // 128], mybir.dt.float32, tag="o")
                    nc.vector.tensor_reduce(out=ot[:], in_=gs[:],
                                            axis=mybir.AxisListType.X,
                                            op=mybir.AluOpType.add)
                    nc.sync.dma_start(out=o_d[b, :, off // 128:(off + n) // 128],
                                      in_=ot[:])
    nc.compile()
    _program_cache[cap] = nc
    return nc


def _install_profile_hook():
    """Register the axon NTFF profile hook (the image's antenv lacks the
    axon_hooks module the boot path would normally use)."""
    import antenv
    if "antenv.axon_hooks" in sys.modules:
        return True
    mod = types.ModuleType("antenv.axon_hooks")
    _hook = [None]
    mod.set_axon_ntff_profile_hook = lambda h: _hook.__setitem__(0, h)
    mod.get_axon_ntff_profile_hook = lambda: _hook[0]
    sys.modules["antenv.axon_hooks"] = mod
    antenv.axon_hooks = mod
    try:
        if "/root/.axon_site" not in sys.path:
            sys.path.insert(0, "/root/.axon_site")
        from trn_agent_boot.trn_boot import _ntff_profile_via_ctypes
        mod.set_axon_ntff_profile_hook(_ntff_profile_via_ctypes("/opt/axon/libaxon_pjrt.so"))
        return True
    except Exception:
        return False


def _wrap16(arr2d):
    """[B, cap] -> [B, 128, cap//16]: index j of each row at [j%16, j//16],
    replicated across the 8 groups of 16 partitions (one per Q7 core)."""
    b, cap = arr2d.shape
    w = arr2d.reshape(b, cap // 16, 16).transpose(0, 2, 1)  # [B, 16, cap//16]
    return np.tile(w, (1, 8, 1)).copy()


def kernel(x, senders, receivers):
    global LAST_EXEC_NS
    x = np.ascontiguousarray(np.asarray(x, dtype=np.float32))
    s = np.asarray(senders).astype(np.int32)
    r = np.asarray(receivers).astype(np.int32)
    n_edges = s.shape[0]
    epc = -(-n_edges // NCORES)

    xp = np.zeros((N_NODES, STEP), np.float32)
    xp[:, :D_FEAT] = x

    percore = []
    max_count = 0
    for c in range(NCORES):
        sc = s[c * epc:(c + 1) * epc]
        rc = r[c * epc:(c + 1) * epc]
        bucket = (sc // RANGE) * NRANGE + (rc // RANGE)
        order = np.argsort(bucket, kind="stable")
        counts = np.bincount(bucket, minlength=NBUCKET)
        max_count = max(max_count, int(counts.max()))
        percore.append((sc, rc, order, counts))

    cap = max(DEFAULT_CAP, round_up_to_multiple(max_count, 1024))
    nc = _build_program(cap)

    in_maps = []
    for c in range(NCORES):
        sc, rc, order, counts = percore[c]
        sl = np.zeros((NBUCKET, cap), np.int16)
        rl = np.zeros((NBUCKET, cap), np.int16)
        ss = (sc[order] % RANGE).astype(np.int16)
        rs = (rc[order] % RANGE).astype(np.int16)
        bounds = np.concatenate(([0], np.cumsum(counts)))
        for b in range(NBUCKET):
            cnt = counts[b]
            sl[b, :cnt] = ss[bounds[b]:bounds[b + 1]]
            rl[b, :cnt] = rs[bounds[b]:bounds[b + 1]]
        in_maps.append({"x": xp, "sidx": _wrap16(sl), "ridx": _wrap16(rl)})

    trace = bool(os.environ.get("KERNEL_TRACE")) and _install_profile_hook()
    res = run_bass_kernel_spmd(nc, in_maps, list(range(NCORES)), trace=trace)
    LAST_EXEC_NS = res.exec_time_ns

    out = np.empty(epc * NCORES, np.float32)
    for c in range(NCORES):
        sc, rc, order, counts = percore[c]
        dev = res.results[c]["out"]  # [NBUCKET, 128, cap//128]
        vals = np.concatenate(
            [dev[b].T.ravel()[:counts[b]] for b in range(NBUCKET)])
        oc = out[c * epc:(c + 1) * epc]
        oc[order] = vals
    return out[:n_edges]
